# revision 1
# baseline (speedup 1.0000x reference)
"""Trainium2 Bass kernel for nn_CubicModelLarge (3-layer cubic-feature MLP).

Strategy: tensor-parallel over the cubic multiplier index i (64 values, 8 per
core).  The cubic expansion is never materialized.  Per layer:

  y[b,o] = W_lin@x + b + sum_t W_sq[o,t] xsq[b,t] + sum_i x[b,i] sum_t W_cu[o,i,t] xsq[b,t]

Rewritten per core c (i in I_c = [8c, 8c+8)):

  H[b,(il,o)] = sum_J F[J,b] * Wcub[J,(il,o)]     (one fp16 GEMM, J = 2176 rows)
  y_c[b,o]    = lin[b,o] + sum_il xmac[b,il] * H[b,(il,o)]
  y = AllReduce_c(y_c)

F rows: 2048 rotation products x_a*x_{(a+d)%64} (d=0..31), 64 x rows (carries
the symmetrized W_sq fold, sharded over i via the x_i scaling), 64 gap-32
products (halved).  Rotated copies of xT are built with PE selection matmuls
(fp16); products on DVE straight out of PSUM into fp16 SBUF; the i-contraction
is fused scalar_tensor_tensor MACs with per-partition scalars.  All GEMM
operands are fp16 (PSUM accumulation fp32) — fp16 over bf16 because the
values here span a tiny dynamic range and the 10-bit mantissa keeps the
3-layer cubic error compounding ~8x lower.  The inter-layer AllReduce runs on
fp16 activations.  Final layer partials are summed on the host in fp32.
"""

import numpy as np

BF16 = np.float16   # host-side element type for prepped weights (fp16)

D = 64
B = 1024
NCORES = 8
I_PER = D // NCORES          # 8
OUTS = (64, 64, 10)
NKCHUNK = 16                 # rotation chunks (d pairs)
NB = B // 128                # 8 batch chunks

_CACHE = {}


# ---------------------------------------------------------------- host prep --

def _maps():
    iu, ju = np.triu_indices(D)
    tmap = np.zeros((D, D), np.int64)
    tmap[iu, ju] = np.arange(len(iu))
    tmap[ju, iu] = tmap[iu, ju]
    p = np.arange(128)
    rows_t = np.zeros((NKCHUNK, 128), np.int64)
    for k in range(NKCHUNK):
        d = 2 * k + p // 64
        a = p % 64
        rows_t[k] = tmap[a, (a + d) % D]
    d32_t = tmap[np.arange(D), (np.arange(D) + 32) % D]
    return tmap, rows_t, d32_t


def _prep_layer(W, b, out):
    """-> (wcub [NCORES](2176, I_PER*out) fp16, wlin [NCORES](65, out) fp16)"""
    _, rows_t, d32_t = _maps()
    W_lin = W[:, :D]
    W_sq = W[:, D:D + 2080]
    W_cu = W[:, D + 2080:].reshape(out, D, 2080)

    iu, ju = np.triu_indices(D)
    w2 = np.zeros((out, D, D), np.float32)
    half = np.where(iu == ju, 1.0, 0.5).astype(np.float32)
    w2[:, iu, ju] = W_sq * half
    w2[:, ju, iu] = W_sq * half

    rt = rows_t.reshape(-1)
    wcubs, wlins = [], []
    for core in range(NCORES):
        I = np.arange(core * I_PER, (core + 1) * I_PER)
        M = I_PER * out
        wcub = np.zeros((17 * 128, M), np.float32)
        blk = W_cu[:, I, :][:, :, rt]                       # (out, I_PER, 2048)
        wcub[:2048] = blk.transpose(2, 1, 0).reshape(2048, M)
        w2blk = w2[:, I, :]                                 # (out, I_PER, 64)
        wcub[2048:2048 + D] = w2blk.transpose(2, 1, 0).reshape(D, M)
        d32blk = W_cu[:, I, :][:, :, d32_t] / 2
        wcub[2048 + D:] = d32blk.transpose(2, 1, 0).reshape(D, M)
        wcubs.append(np.ascontiguousarray(wcub.astype(BF16)))

        wl = np.zeros((65, out), np.float32)
        if core == 0:
            wl[:D] = W_lin.T
            wl[D] = b
        wlins.append(wl.astype(BF16))
    return wcubs, wlins


def _sel_consts():
    """Selection matrices, concatenated (64, (NKCHUNK+2)*128).

    slot k in 0..15: [rot_{2k}; rot_{2k+1}]   sel[c, k*128 + h*64 + a] = (c == (a + 2k + h) % 64)
    slot 16: [rot0; rot0]  (builds xT2)
    slot 17: [rot32; rot32] (first 64 cols used, builds xd32)
    """
    sel = np.zeros((D, (NKCHUNK + 2) * 128), np.float32)
    for k in range(NKCHUNK):
        for p in range(128):
            d = 2 * k + p // 64
            a = p % 64
            sel[(a + d) % D, k * 128 + p] = 1.0
    for p in range(128):
        sel[p % 64, NKCHUNK * 128 + p] = 1.0
        sel[(p % 64 + 32) % D, (NKCHUNK + 1) * 128 + p] = 1.0
    return sel


# ------------------------------------------------------------------ builder --

def _build_module():
    import concourse.bacc as bacc
    import concourse.mybir as mybir
    import concourse.tile as tile

    F32 = mybir.dt.float32
    F16 = mybir.dt.float16
    MULT = mybir.AluOpType.mult
    ADD = mybir.AluOpType.add

    nc = bacc.Bacc("TRN2", target_bir_lowering=False, num_devices=NCORES, debug=False)

    x_in = nc.dram_tensor("x", [B, D], F32, kind="ExternalInput")
    wcub_in = [
        nc.dram_tensor(f"wcub{li}", [17 * 128, I_PER * OUTS[li]], F16, kind="ExternalInput")
        for li in range(3)
    ]
    wlin_in = [
        nc.dram_tensor(f"wlin{li}", [65, OUTS[li]], F16, kind="ExternalInput")
        for li in range(3)
    ]
    colsel_in = nc.dram_tensor("colsel", [D, I_PER], F16, kind="ExternalInput")
    out_ext = nc.dram_tensor("out", [B, OUTS[2]], F32, kind="ExternalOutput")

    sel_c = nc.inline_tensor(_sel_consts().astype(np.float16), name="selc")
    ident_c = nc.inline_tensor(np.eye(128, dtype=np.float32), name="identc")
    ident16_c = nc.inline_tensor(np.eye(128, dtype=np.float16), name="ident16c")

    with tile.TileContext(nc) as tc:
        with (
            tc.tile_pool(name="wpool", bufs=2) as wpool,
            tc.tile_pool(name="spool", bufs=1) as spool,
            tc.tile_pool(name="xpool", bufs=2) as xpool,
            tc.tile_pool(name="qpool", bufs=1) as qpool,
            tc.tile_pool(name="ypool", bufs=2) as ypool,
            tc.tile_pool(name="ps_rep", bufs=3, space="PSUM") as ps_rep,
            tc.tile_pool(name="ps_h", bufs=2, space="PSUM") as ps_h,
            tc.tile_pool(name="ps_small", bufs=2, space="PSUM") as ps_small,
            tc.tile_pool(name="dpool", bufs=2, space="DRAM") as dpool,
        ):
            sel_sb = spool.tile([D, (NKCHUNK + 2) * 128], F16, tag="sel")
            nc.sync.dma_start(sel_sb[:], sel_c.ap())
            ident_sb = spool.tile([128, 128], F32, tag="ident")
            nc.sync.dma_start(ident_sb[:], ident_c.ap())
            ident16_sb = spool.tile([128, 128], F16, tag="ident16")
            nc.sync.dma_start(ident16_sb[:], ident16_c.ap())
            colsel_sb = spool.tile([D, I_PER], F16, tag="colsel")
            nc.sync.dma_start(colsel_sb[:], colsel_in.ap())

            HB = 512            # half-batch
            NBH = HB // 128     # 4 chunks per half

            # x tiles for layer 0, both halves, straight from the input (fp32)
            # -- issued FIRST and on the sync queue so phase A isn't stuck
            # behind megabytes of weight DMA
            x_half = []
            for h in range(2):
                xs = xpool.tile([128, NBH, D], F32, tag=f"x{h}")
                nc.sync.dma_start(
                    xs[:],
                    x_in.ap()[h * HB:(h + 1) * HB, :]
                    .rearrange("(bc p) f -> p bc f", p=128),
                )
                x_half.append(xs)

            # per-layer weight tiles on the scalar-engine DMA queue; wcub is
            # DMA'd per k-chunk so phase C can start as soon as chunk 0 lands
            # instead of waiting for the full 2.2 MB tile
            weights = []
            for li in range(3):
                M = I_PER * OUTS[li]
                wcub_sb = wpool.tile([128, NKCHUNK, M], F16, tag="wcub")
                src = wcub_in[li].ap()[: 16 * 128, :].rearrange(
                    "(k p) m -> p k m", p=128
                )
                for k in range(NKCHUNK):
                    nc.scalar.dma_start(wcub_sb[:, k, :], src[:, k, :])
                wx_sb = wpool.tile([D, M], F16, tag="wx")
                nc.scalar.dma_start(wx_sb[:], wcub_in[li].ap()[2048:2048 + D, :])
                wd32_sb = wpool.tile([D, M], F16, tag="wd32")
                nc.scalar.dma_start(wd32_sb[:], wcub_in[li].ap()[2048 + D:, :])
                wlin_sb = wpool.tile([65, OUTS[li]], F16, tag="wlin")
                nc.scalar.dma_start(wlin_sb[:], wlin_in[li].ap())
                weights.append((wcub_sb, wx_sb, wd32_sb, wlin_sb))

            for li in range(3):
                out_l = OUTS[li]
                M = I_PER * out_l
                last = li == 2
                wcub_sb, wx_sb, wd32_sb, wlin_sb = weights[li]
                next_x = [None, None]

                for h in range(2):
                    x_sb = x_half[h]
                    idt = ident_sb if li == 0 else ident16_sb

                    # -- phase A
                    xT_sb = xpool.tile([65, HB], F16, tag=f"xT{h}")
                    for bc in range(NBH):
                        if li == 0:
                            xTp = ps_small.tile([D, 128], F32, tag="small")
                        else:
                            xTp = ps_small.tile([D, 128], F16, tag="smallT", bufs=1)
                        nc.tensor.transpose(xTp[:], x_sb[:, bc, :], idt[:])
                        nc.scalar.copy(xT_sb[0:D, bc * 128:(bc + 1) * 128], xTp[:])
                    nc.vector.memset(xT_sb[D:65, :], 1.0)

                    xT2_sb = xpool.tile([128, HB], F16, tag=f"xT2{h}")
                    rep00 = ps_rep.tile([128, HB], F32, tag="rep")
                    nc.tensor.matmul(
                        rep00[:], sel_sb[:, NKCHUNK * 128:(NKCHUNK + 1) * 128],
                        xT_sb[0:D, :], start=True, stop=True,
                    )
                    nc.scalar.copy(xT2_sb[:], rep00[:])

                    xd32_sb = xpool.tile([D, HB], F16, tag=f"xd32{h}")
                    rep32 = ps_rep.tile([128, HB], F32, tag="rep")
                    nc.tensor.matmul(
                        rep32[:], sel_sb[:, (NKCHUNK + 1) * 128:(NKCHUNK + 2) * 128],
                        xT_sb[0:D, :], start=True, stop=True,
                    )
                    nc.vector.tensor_mul(xd32_sb[:], xT2_sb[0:D, :], rep32[0:D, :])

                    # -- phase B: rep matmul (PE) -> product (DVE, PSUM read)
                    xsq = []
                    for k in range(NKCHUNK):
                        rep = ps_rep.tile([128, HB], F32, tag="rep")
                        nc.tensor.matmul(
                            rep[:], sel_sb[:, k * 128:(k + 1) * 128],
                            xT_sb[0:D, :], start=True, stop=True,
                        )
                        xq = qpool.tile([128, HB], F16, tag=f"xsq{k}h{h}")
                        nc.vector.tensor_mul(xq[:], xT2_sb[:], rep[:])
                        xsq.append(xq)

                    # -- phase C
                    y_sb = ypool.tile(
                        [128, NBH, out_l], F32 if last else F16, tag=f"y{li}{h}"
                    )
                    if not last:
                        for bc in range(NBH):
                            bs = slice(bc * 128, (bc + 1) * 128)
                            h_ps = ps_h.tile([128, M], F32, tag="h")
                            for k in range(NKCHUNK):
                                nc.tensor.matmul(
                                    h_ps[:], xsq[k][:, bs], wcub_sb[:, k, :],
                                    start=(k == 0), stop=False,
                                )
                            nc.tensor.matmul(h_ps[:], xT_sb[0:D, bs], wx_sb[:], start=False, stop=False)
                            nc.tensor.matmul(h_ps[:], xd32_sb[:, bs], wd32_sb[:], start=False, stop=True)

                            lin_ps = ps_small.tile([128, out_l], F32, tag="small")
                            nc.tensor.matmul(lin_ps[:], xT_sb[0:65, bs], wlin_sb[:], start=True, stop=True)
                            xmac_ps = ps_small.tile([128, I_PER], F32, tag="small")
                            nc.tensor.matmul(xmac_ps[:], xT_sb[0:D, bs], colsel_sb[:], start=True, stop=True)
                            xmac_sb = ypool.tile([128, I_PER], F32, tag="xmac")
                            nc.scalar.copy(xmac_sb[:], xmac_ps[:])

                            nc.scalar.copy(y_sb[:, bc, :], lin_ps[:])
                            for il in range(I_PER):
                                nc.vector.scalar_tensor_tensor(
                                    y_sb[:, bc, :],
                                    h_ps[:, il * out_l:(il + 1) * out_l],
                                    xmac_sb[:, il:il + 1],
                                    y_sb[:, bc, :],
                                    op0=MULT, op1=ADD,
                                )

                        # -- phase D: AllReduce this half (fp16 activations)
                        y_bounce = dpool.tile([HB, out_l], F16, tag=f"ybounce{h}")
                        y_red = dpool.tile([HB, out_l], F16, tag=f"yred{h}")
                        nc.sync.dma_start(
                            y_bounce[:].rearrange("(bc p) o -> p bc o", p=128), y_sb[:]
                        )
                        nc.gpsimd.collective_compute(
                            "AllReduce",
                            ADD,
                            replica_groups=[list(range(NCORES))],
                            ins=[y_bounce.opt()],
                            outs=[y_red.opt()],
                        )
                        xs = xpool.tile([128, NBH, D], F16, tag=f"xr{li}{h}")
                        nc.sync.dma_start(
                            xs[:], y_red[:].rearrange("(bc p) f -> p bc f", p=128)
                        )
                        next_x[h] = xs
                    else:
                        # layer 2: stationary-W GEMM, transpose, MAC
                        h_ps = ps_h.tile([M, HB], F32, tag="h")
                        for k in range(NKCHUNK):
                            nc.tensor.matmul(
                                h_ps[:], wcub_sb[:, k, :], xsq[k][:],
                                start=(k == 0), stop=False,
                            )
                        nc.tensor.matmul(h_ps[:], wx_sb[:], xT_sb[0:D, :], start=False, stop=False)
                        nc.tensor.matmul(h_ps[:], wd32_sb[:], xd32_sb[:], start=False, stop=True)
                        h2_sb = ypool.tile([M, HB], F32, tag=f"h2{h}")
                        nc.scalar.copy(h2_sb[:], h_ps[:])

                        for bc in range(NBH):
                            bs = slice(bc * 128, (bc + 1) * 128)
                            lin_ps = ps_small.tile([128, out_l], F32, tag="small")
                            nc.tensor.matmul(lin_ps[:], xT_sb[0:65, bs], wlin_sb[:], start=True, stop=True)
                            xmac_ps = ps_small.tile([128, I_PER], F32, tag="small")
                            nc.tensor.matmul(xmac_ps[:], xT_sb[0:D, bs], colsel_sb[:], start=True, stop=True)
                            xmac_sb = ypool.tile([128, I_PER], F32, tag="xmac")
                            nc.scalar.copy(xmac_sb[:], xmac_ps[:])
                            nc.scalar.copy(y_sb[:, bc, :], lin_ps[:])

                            h2t_ps = ps_small.tile([128, M], F32, tag="small")
                            nc.tensor.transpose(h2t_ps[:], h2_sb[:, bs], ident_sb[0:M, 0:M])
                            for il in range(I_PER):
                                nc.vector.scalar_tensor_tensor(
                                    y_sb[:, bc, :],
                                    h2t_ps[:, il * out_l:(il + 1) * out_l],
                                    xmac_sb[:, il:il + 1],
                                    y_sb[:, bc, :],
                                    op0=MULT, op1=ADD,
                                )

                        nc.sync.dma_start(
                            out_ext.ap()[h * HB:(h + 1) * HB, :]
                            .rearrange("(bc p) o -> p bc o", p=128),
                            y_sb[:],
                        )

                if not last:
                    x_half = next_x

    nc.compile()
    return nc


# ------------------------------------------------------------------- runner --

def kernel(x, W0, b0, W1, b1, W2, b2):
    from concourse.bass_utils import run_bass_kernel_spmd

    if "nc" not in _CACHE:
        _CACHE["nc"] = _build_module()
    nc = _CACHE["nc"]

    x = np.ascontiguousarray(np.asarray(x, np.float32))
    Ws = [np.asarray(W, np.float32) for W in (W0, W1, W2)]
    bs = [np.asarray(b_, np.float32) for b_ in (b0, b1, b2)]

    wcubs, wlins = {}, {}
    for li in range(3):
        wcubs[li], wlins[li] = _prep_layer(Ws[li], bs[li], OUTS[li])

    in_maps = []
    for core in range(NCORES):
        I = np.arange(core * I_PER, (core + 1) * I_PER)
        colsel = np.zeros((D, I_PER), np.float32)
        colsel[I, np.arange(I_PER)] = 1.0
        m = {"x": x, "colsel": colsel.astype(BF16)}
        for li in range(3):
            m[f"wcub{li}"] = wcubs[li][core]
            m[f"wlin{li}"] = wlins[li][core]
        in_maps.append(m)

    res = run_bass_kernel_spmd(nc, in_maps, core_ids=list(range(NCORES)))
    out = np.zeros((B, OUTS[2]), np.float32)
    for core in range(NCORES):
        out += res.results[core]["out"]
    return out

